# revision 27
# baseline (speedup 1.0000x reference)
"""Binarized ResNet BasicBlock (2x binarized 3x3 conv + batchnorm + hardtanh,
residual) on 8 Trainium2 NeuronCores, data-parallel over batch.

Math (per reference):
  s1  = conv3x3(sign(x), sign(W1), pad=1)          # integer-valued
  h   = clip(bn1(s1), -1, 1)                       # only sign(h) is consumed
  s2p = conv3x3(sign(h), sign(W2), pad=1) + x
  out = clip(bn2(s2p), -1, 1)

Key points:
  - sign(h) = sign(a1*s1 + c1) per channel (a1 = g1*rsqrt(v1+eps),
    c1 = b1 - m1*a1), so h is never materialized.
  - batchnorm needs global batch stats: each core computes per-channel
    (E[x], E[x^2]) partials over its 4 images; tiny AllReduces combine them
    (equal pixel counts per core, so mean-of-means works). Each barrier's
    AllReduce is split in two: {pc0, pc1} issued right after their last conv
    group (hidden under the remaining pc2 conv work), {pc2} at the end (the
    only exposed latency).
  - fp8 mode (default): +/-1 activations/weights in fp8e4 are exact; the
    3x3 conv's 27 (channel-chunk, tap) units are packed into 13 DoubleRow
    K=256 matmuls + 1 normal K=128 matmul per output tile. The rhs must be
    a depth-2 AP (pair dim + one run), so each tile reads contiguous
    464-wide runs of the 58-col padded plane and the evacuation strips the
    2-col seam. Pair base addresses must be 2B-aligned and pair strides
    16B-aligned, so shifted copies of the sign plane provide the dx=1 taps
    (shift +1 col) and a row-shifted plane pairs up the leftover dx=2 taps.
  - s1 and s2p stay resident in SBUF as fp16 (integers < 2048: exact; s2p
    adds the fp32 residual, fp16 rounding ~5e-4 relative).
  - lead-in: image 0 is loaded chunk-major with banded shift copies and its
    conv1 split in two chunk-sets, so the first matmul starts after ~2/7 of
    the image instead of the whole image. Plane buffers get border-only
    memsets (the shift copies cover B/X2 planes entirely).
  - pass C writes pc0/pc1 outputs first: their scale/bias comes from the
    early AllReduce, so the output DMA starts at conv2 end, overlapping the
    exposed pc2 AllReduce.
"""

import contextlib

import numpy as np
import ml_dtypes

import concourse.bass as bass
import concourse.tile as tile
from concourse import bacc, mybir
from concourse.bass_utils import run_bass_kernel_spmd
from concourse.replica_groups import maybe_share_collective_output_space

F32 = mybir.dt.float32
F16 = mybir.dt.float16
BF16 = mybir.dt.bfloat16
F8 = mybir.dt.float8e4
F8NP = mybir.dt.np(F8)

NCORES = 8
B, C, H, W = 32, 384, 56, 56
P = C
BPC = B // NCORES         # images per core
NCC = C // 128            # input channel chunks
NPC = P // 128            # output channel chunks
HP = H + 2                # padded rows
WP = W + 2                # padded cols
PADPIX = HP * WP          # 3364
NPIX = H * W              # 3136
CHUNK_ROWS = 8            # output rows per PSUM tile
NCHUNK = H // CHUNK_ROWS  # 7
CHW = CHUNK_ROWS * W      # 448
EPS = 1e-5

MODE = "fp8"              # "bf16" or "fp8"
FP8_SWIL = False          # plain DoubleRow beat SwInterleave on HW
SHIFT_ENGINE = "dma"      # "dma" | "gpsimd" | "vector": shifted-plane copies
FINE_COPIES = True        # per-plane shift copies (finer deps; A/B: -25us)
DEEP_BUFS = True          # xin/xr/oc staging depth 8 instead of 6
SPLIT_AR = True           # split each stats AllReduce into {pc0,pc1} + {pc2}
EARLY_IMG0 = True         # chunk-major img0 load + banded copies + 2 chunk sets
CSTRIDE = 3376            # fp8 padded plane stride (16B-aligned)
RUN = CHUNK_ROWS * WP     # 464 <= 512: contiguous rhs run incl. seam
NPLANE = 7                # A0 B0 A1 B1 A2 B2 X2

# fp8 unit schedule: 13 DoubleRow pairs + 1 single cover the 27 (cc, dy, dx)
# conv units. Planes: A-cc at 2cc (shift 0), B-cc at 2cc+1 (shift +1 col,
# provides dx=1 taps at even base addresses), X2 at 6 (A2 shifted one row,
# provides the dy+1 partner for cc2 dx=2 taps).
#  dx01 pair (cc, dy): taps (cc,dy,0)@A-cc, (cc,dy,1)@B-cc;
#    rhs sx[:, 2cc:2cc+2, q:q+RUN], q=(y0+dy)*WP
#  cc01 pair (dy): taps (0,dy,2)@A0, (1,dy,2)@A1;
#    rhs sx[:, 0:3:2, q:q+RUN], q=(y0+dy)*WP+2
#  xp pair: taps (2,0,2)@A2, (2,1,2)@X2; rhs sx[:, 4:7:2, q:q+RUN],
#    q=y0*WP+2
#  single: tap (2,2,2)@A2; rhs sx[:, 4, q:q+RUN], q=(y0+2)*WP+2
FP8_PAIRS = (
    [("dx01", cc, dy) for dy in range(3) for cc in range(3)]
    + [("cc01", None, dy) for dy in range(3)]
    + [("xp", None, None)]
)
NUNIT_FP8 = len(FP8_PAIRS) + 1  # 14

# AllReduce pc grouping. bn1: one AR (conv2 needs every pc's threshold, so
# splitting only serializes collectives for nothing). bn2: group A = pc 0..1
# issued one conv group early (its scale/bias lands by conv2 end, so 2/3 of
# the output store overlaps the exposed group-B AllReduce).
AR_GROUPS1 = [(0, NPC)]
AR_GROUPS2 = [(0, 2), (2, 3)] if SPLIT_AR else [(0, NPC)]


def _prep_weight(w):
    """bf16 mode: [P, C, 3, 3] -> [NCC, 128, 9*NPC*128] bf16 sign; lhsT for
    (cc, off, pc) is wsb[cc][:, (off*NPC+pc)*128 : +128]."""
    ws = np.sign(w.astype(np.float32)).astype(ml_dtypes.bfloat16)
    arr = ws.transpose(1, 2, 3, 0).reshape(C, 9, P)
    arr = arr.reshape(NCC, 128, 9, NPC, 128)
    return np.ascontiguousarray(arr.reshape(NCC, 128, 9 * NPC * 128))


def _fp8_pair_units():
    """(uA, uB) tap indices per FP8_PAIRS entry; each tap is (cc, dy, dx)."""
    out = []
    for kind, cc, dy in FP8_PAIRS:
        if kind == "dx01":
            out.append(((cc, dy, 0), (cc, dy, 1)))
        elif kind == "cc01":
            out.append(((0, dy, 2), (1, dy, 2)))
        else:
            out.append(((2, 0, 2), (2, 1, 2)))
    return out


def _prep_weight_fp8(w):
    """[P, C, 3, 3] -> (pairs [128, 13*NPC*256], single [128, NPC*128]) fp8
    sign values. SwInterleave layout: per pair/pc block of 256, columns are
    [A127, B127, A126, B126, ..., A0, B0] (interleaved, reversed)."""
    ws = np.sign(w.astype(np.float32))
    arr = ws.transpose(1, 2, 3, 0).reshape(NCC, 128, 3, 3, NPC, 128)

    def unit(cc, dy, dx):  # [128 (c), NPC, 128 (m)]
        return arr[cc, :, dy, dx]

    npair = len(FP8_PAIRS)
    if FP8_SWIL:
        wp = np.zeros((128, npair, NPC, 256), np.float32)
        for j, (uA, uB) in enumerate(_fp8_pair_units()):
            wp[:, j, :, 0::2] = unit(*uA)[:, :, ::-1]
            wp[:, j, :, 1::2] = unit(*uB)[:, :, ::-1]
    else:
        wp = np.zeros((128, npair, NPC, 2, 128), np.float32)
        for j, (uA, uB) in enumerate(_fp8_pair_units()):
            wp[:, j, :, 0] = unit(*uA)
            wp[:, j, :, 1] = unit(*uB)
    wsg = unit(2, 2, 2)  # [128, NPC, 128]
    return (
        np.ascontiguousarray(wp.reshape(128, -1)).astype(F8NP),
        np.ascontiguousarray(wsg.reshape(128, -1)).astype(F8NP),
    )


def _prep_vecs(g1, b1, g2, b2):
    """-> [128, NPC, 4] f32: per-partition (p_in) per-chunk (pc) gamma/beta."""
    out = np.empty((128, NPC, 4), np.float32)
    for k, v in enumerate((g1, b1, g2, b2)):
        out[:, :, k] = v.astype(np.float32).reshape(NPC, 128).T
    return out


def _stats_to_scale_bias(nc, singles, allout, vecs_sb, eps_tile, gk, bk, name,
                         ncores, width):
    """allout [128, width, 2] summed (E, E2) over cores -> a, c [128, width, 1].
    vecs_sb is the matching [128, width, 4] slice."""
    g = width
    Eg = singles.tile([128, g, 1], F32, name=f"{name}_Eg")
    E2g = singles.tile([128, g, 1], F32, name=f"{name}_E2g")
    var = singles.tile([128, g, 1], F32, name=f"{name}_var")
    tmp = singles.tile([128, g, 1], F32, name=f"{name}_tmp")
    sd = singles.tile([128, g, 1], F32, name=f"{name}_sd")
    rs = singles.tile([128, g, 1], F32, name=f"{name}_rs")
    a = singles.tile([128, g, 1], F32, name=f"{name}_a")
    c = singles.tile([128, g, 1], F32, name=f"{name}_c")
    nc.scalar.mul(Eg[:], allout[:, :, 0:1], 1.0 / ncores)
    nc.scalar.mul(E2g[:], allout[:, :, 1:2], 1.0 / ncores)
    nc.vector.tensor_mul(tmp[:], Eg[:], Eg[:])
    nc.vector.tensor_tensor(
        out=var[:], in0=E2g[:], in1=tmp[:], op=mybir.AluOpType.subtract
    )
    nc.scalar.activation(
        sd[:], var[:], mybir.ActivationFunctionType.Sqrt, bias=eps_tile[:],
        scale=1.0,
    )
    nc.vector.reciprocal(out=rs[:], in_=sd[:])
    nc.vector.tensor_mul(a[:], rs[:], vecs_sb[:, :, gk : gk + 1])
    nc.vector.tensor_mul(tmp[:], Eg[:], a[:])
    nc.vector.tensor_tensor(
        out=c[:], in0=vecs_sb[:, :, bk : bk + 1], in1=tmp[:],
        op=mybir.AluOpType.subtract,
    )
    return a, c


def _emit_conv_bf16(nc, psum_pool, wsb, sx_tile, pc, chunk):
    """27 accumulated bf16 matmuls -> psum tile [128, CHW]."""
    ps = psum_pool.tile([128, CHW], F32, name="ps", tag="ps")
    y0 = chunk * CHUNK_ROWS
    k = 0
    for cc in range(NCC):
        sx3 = sx_tile[:, cc, :].rearrange("p (h w) -> p h w", w=WP)
        for off in range(9):
            dy, dx = off // 3, off % 3
            lhsT = wsb[cc][:, (off * NPC + pc) * 128 : (off * NPC + pc + 1) * 128]
            rhs = sx3[:, y0 + dy : y0 + dy + CHUNK_ROWS, dx : dx + W]
            nc.tensor.matmul(
                ps[:], lhsT, rhs, start=(k == 0), stop=(k == 9 * NCC - 1)
            )
            k += 1
    return ps


def _emit_conv_fp8(nc, psum_pool, wp_view, ws_view, sx_tile, pc,
                   chunk_sets=None):
    """Weight-stationary fp8 DoubleRow conv for one (img, pc): returns NCHUNK
    psum tiles [128, RUN]; valid output cols = (8, 58) view sliced [:, :56].
    By default all 7 chunks accumulate in one weight-stationary pass (7 of 8
    PSUM banks; splitting into half-groups measured slower for steady-state
    images -- the extra DoubleRow LDWEIGHTS cost more than the bank stalls
    they saved). chunk_sets overrides the grouping (used for image 0's
    lead-in, where starting on the first rows early wins)."""
    perf = (mybir.MatmulPerfMode.DoubleRowSwInterleave if FP8_SWIL
            else mybir.MatmulPerfMode.DoubleRow)
    if chunk_sets is None:
        chunk_sets = [range(0, NCHUNK)]
    pss = {}
    for cset in chunk_sets:
        for chunk in cset:
            pss[chunk] = psum_pool.tile([128, RUN], F32, name="ps", tag="ps")
        u = 0
        for j, (kind, cc, dy) in enumerate(FP8_PAIRS):
            lhsT = wp_view[:, j, pc]
            for chunk in cset:
                y0 = chunk * CHUNK_ROWS
                if kind == "dx01":
                    q = (y0 + dy) * WP
                    rhs = sx_tile[:, 2 * cc : 2 * cc + 2, q : q + RUN]
                elif kind == "cc01":
                    q = (y0 + dy) * WP + 2
                    rhs = sx_tile[:, 0:3:2, q : q + RUN]
                else:  # xp
                    q = y0 * WP + 2
                    rhs = sx_tile[:, 4:7:2, q : q + RUN]
                nc.tensor.matmul(
                    pss[chunk][:], lhsT, rhs,
                    start=(u == 0), stop=(u == NUNIT_FP8 - 1), perf_mode=perf,
                )
            u += 1
        lhsT = ws_view[:, pc]
        for chunk in cset:
            y0 = chunk * CHUNK_ROWS
            q = (y0 + 2) * WP + 2
            rhs = sx_tile[:, 4, q : q + RUN]
            nc.tensor.matmul(
                pss[chunk][:], lhsT, rhs,
                start=(u == 0), stop=(u == NUNIT_FP8 - 1),
            )
    return [pss[c] for c in range(NCHUNK)]


def build_program(bpc=BPC, ncores=NCORES, mode=MODE, timing_iters=None):
    nc = bacc.Bacc(
        "TRN2",
        target_bir_lowering=False,
        debug=False,
        enable_asserts=True,
        num_devices=ncores,
    )
    x_d = nc.dram_tensor("x", [bpc, C, H, W], F32, kind="ExternalInput").ap()
    if mode == "bf16":
        w1_d = nc.dram_tensor("w1", [NCC, 128, 9 * NPC * 128], BF16,
                              kind="ExternalInput").ap()
        w2_d = nc.dram_tensor("w2", [NCC, 128, 9 * NPC * 128], BF16,
                              kind="ExternalInput").ap()
    else:
        wpair_elems = len(FP8_PAIRS) * NPC * 256
        w1p_d = nc.dram_tensor("w1p", [128, wpair_elems], F8,
                               kind="ExternalInput").ap()
        w1s_d = nc.dram_tensor("w1s", [128, NPC * 128], F8,
                               kind="ExternalInput").ap()
        w2p_d = nc.dram_tensor("w2p", [128, wpair_elems], F8,
                               kind="ExternalInput").ap()
        w2s_d = nc.dram_tensor("w2s", [128, NPC * 128], F8,
                               kind="ExternalInput").ap()
    vecs_d = nc.dram_tensor("vecs", [128, NPC, 4], F32,
                            kind="ExternalInput").ap()
    out_d = nc.dram_tensor("out", [bpc, C, H, W], F32,
                           kind="ExternalOutput").ap()

    with tile.TileContext(nc) as tc:
        with (
            tc.tile_pool(name="weights",
                         bufs=NCC if mode == "bf16" else 2) as wpool,
            tc.tile_pool(name="singles", bufs=1) as singles,
            tc.tile_pool(name="sx", bufs=1) as sxpool,
            tc.tile_pool(name="acc", bufs=3 * bpc) as accpool,
            tc.tile_pool(name="xin", bufs=8 if DEEP_BUFS else 6) as xinpool,
            tc.tile_pool(name="xin0", bufs=2) as xin0pool,
            tc.tile_pool(name="xr", bufs=8 if DEEP_BUFS else 6) as xrpool,
            tc.tile_pool(name="oc", bufs=8 if DEEP_BUFS else 6) as ocpool,
            tc.tile_pool(name="stats", bufs=1) as stpool,
            tc.tile_pool(name="psum", bufs=8, space="PSUM") as psum_pool,
            tc.tile_pool(name="dram", bufs=1, space="DRAM") as dram,
        ):
            # ---- constants (outside the timing loop) ----
            if mode == "bf16":
                # W1/W2 share slots (tag "w"): W2 allocates after conv1
                # releases W1, overlapping the bn1 AllReduce.
                wsb1 = []
                for cc in range(NCC):
                    t1 = wpool.tile([128, 9 * NPC * 128], BF16,
                                    name=f"w1sb{cc}", tag="w")
                    nc.sync.dma_start(out=t1, in_=w1_d[cc])
                    wsb1.append(t1)
            else:
                w1p_sb = wpool.tile([128, len(FP8_PAIRS) * NPC * 256], F8,
                                    name="w1p_sb", tag="wp")
                nc.sync.dma_start(out=w1p_sb, in_=w1p_d)
                w1s_sb = wpool.tile([128, NPC * 128], F8, name="w1s_sb",
                                    tag="ws")
                nc.sync.dma_start(out=w1s_sb, in_=w1s_d)
                # w2p/w2s DMAs are deferred into the pass-A loop so they do
                # not sit ahead of image 0's loads in the HWDGE FIFO.
                w2p_sb = wpool.tile([128, len(FP8_PAIRS) * NPC * 256], F8,
                                    name="w2p_sb", tag="wp")
                w2s_sb = wpool.tile([128, NPC * 128], F8, name="w2s_sb",
                                    tag="ws")
                if FP8_SWIL:
                    w1p_v = w1p_sb.rearrange("p (j q m) -> p j q m",
                                             j=len(FP8_PAIRS), q=NPC)
                    w2p_v = w2p_sb.rearrange("p (j q m) -> p j q m",
                                             j=len(FP8_PAIRS), q=NPC)
                else:
                    w1p_v = w1p_sb.rearrange("p (j q i m) -> p j q i m",
                                             j=len(FP8_PAIRS), q=NPC, i=2)
                    w2p_v = w2p_sb.rearrange("p (j q i m) -> p j q i m",
                                             j=len(FP8_PAIRS), q=NPC, i=2)
                w1s_v = w1s_sb.rearrange("p (q m) -> p q m", q=NPC)
                w2s_v = w2s_sb.rearrange("p (q m) -> p q m", q=NPC)
            vecs_sb = singles.tile([128, NPC, 4], F32)
            nc.sync.dma_start(out=vecs_sb, in_=vecs_d)
            eps_tile = singles.tile([128, 1], F32)
            nc.vector.memset(eps_tile, EPS)

            # persistent padded sign planes (borders stay zero forever)
            sxt = []
            for s in range(2):
                if mode == "bf16":
                    t = sxpool.tile([128, NCC, PADPIX], BF16, name=f"sx{s}")
                    nc.gpsimd.memset(t[:], 0.0)
                else:
                    t = sxpool.tile([128, NPLANE, CSTRIDE], F8, name=f"sx{s}")
                    # Only A planes need zero borders: the sign writes cover
                    # rows 1..56 cols 1..56; shift copies fill B/X2 planes
                    # entirely from A (plus A's borders). Memset only the
                    # border strips, split across idle engines.
                    for icc in range(NCC):
                        pl = 2 * icc
                        eng = (nc.vector, nc.gpsimd)[icc % 2]
                        pv = t[:, pl, :]
                        eng.memset(pv[0:128, 0:WP], 0.0)
                        eng.memset(pv[0:128, (HP - 1) * WP : CSTRIDE], 0.0)
                        # cols 0 and 57 for rows 1..56, one strided AP
                        colv = t[:, pl, WP : WP + (H * WP)].rearrange(
                            "p (h w) -> p h w", w=WP)
                        eng.memset(colv[:, :, 0 : WP : WP - 1], 0.0)
                sxt.append(t)

            bnst1 = [
                stpool.tile([128, bpc * NCHUNK, 6], F32, name=f"bnst1_{pc}")
                for pc in range(NPC)
            ]
            bnst2 = [
                stpool.tile([128, bpc * NCHUNK, 6], F32, name=f"bnst2_{pc}")
                for pc in range(NPC)
            ]

            cc_addr_space = (
                "Local" if timing_iters is not None
                else maybe_share_collective_output_space(
                    "AllReduce", [list(range(ncores))]
                )
            )

            def do_allreduce(cin, cout):
                if timing_iters is None:
                    nc.gpsimd.collective_compute(
                        "AllReduce",
                        mybir.AluOpType.add,
                        replica_groups=[list(range(ncores))],
                        ins=[cin.opt()],
                        outs=[cout.opt()],
                    )
                else:
                    nc.gpsimd.dma_start(out=cout, in_=cin)

            def emit_group_stats(bnst, pcs, tag):
                """bn_aggr + (E, E2) combine for pcs -> DRAM -> AllReduce.
                Returns (cout_dram, width)."""
                g = len(pcs)
                allin = singles.tile([128, g, 2], F32, name=f"allin_{tag}")
                for k, pc in enumerate(pcs):
                    mv = stpool.tile([128, 2], F32, name=f"mv_{tag}_{pc}")
                    nc.vector.bn_aggr(out=mv, in_=bnst[pc])
                    nc.vector.tensor_copy(allin[:, k, 0:1], mv[:, 0:1])
                    sq = stpool.tile([128, 1], F32, name=f"sq_{tag}_{pc}")
                    nc.vector.tensor_mul(sq, mv[:, 0:1], mv[:, 0:1])
                    nc.vector.tensor_tensor(
                        out=allin[:, k, 1:2], in0=mv[:, 1:2], in1=sq,
                        op=mybir.AluOpType.add,
                    )
                cin = dram.tile([128, g * 2], F32, name=f"cc_{tag}_in")
                cout = dram.tile([128, g * 2], F32, name=f"cc_{tag}_out",
                                 addr_space=cc_addr_space)
                nc.sync.dma_start(
                    out=cin, in_=allin.rearrange("p a b -> p (a b)"))
                do_allreduce(cin, cout)
                return cout, g

            def finish_group_stats(cout, g, gk, bk, tag, pc0):
                """DRAM result -> SBUF -> per-channel scale/bias [128, g, 1]."""
                allout = singles.tile([128, g, 2], F32, name=f"allout_{tag}")
                # sync ring (NOT gpsimd: a readback there would sit between
                # the two collectives in the gpsimd FIFO and serialize them).
                # Emission order keeps anything urgent from queuing behind it.
                nc.sync.dma_start(
                    out=allout.rearrange("p a b -> p (a b)"), in_=cout)
                vs = vecs_sb[:, pc0 : pc0 + g, :]
                return _stats_to_scale_bias(
                    nc, singles, allout, vs, eps_tile, gk, bk, tag, ncores, g)

            def make_shift_copies(sx_tile, band=None, engine=None):
                """B planes = A planes shifted +1 col; X2 = A2 shifted one
                row. Pad regions supply the zeros. band=(r0, r1) restricts
                the copy to plane rows [r0, r1) (X2 gets [r0, r1-1) capped)."""
                eng = engine or SHIFT_ENGINE
                if band is None:
                    b0, b1 = 0, CSTRIDE // WP  # rows 0..58
                    lo, hi = 0, CSTRIDE - 1
                    xlo, xhi = 0, CSTRIDE - WP
                else:
                    b0, b1 = band
                    lo, hi = b0 * WP, min(b1 * WP, CSTRIDE - 1)
                    xlo, xhi = b0 * WP, min((b1 - 1) * WP, CSTRIDE - WP)
                if FINE_COPIES:
                    pairs = [
                        (sx_tile[:, 2 * cc + 1, lo:hi],
                         sx_tile[:, 2 * cc, lo + 1 : hi + 1])
                        for cc in range(NCC)
                    ]
                else:
                    pairs = [
                        (sx_tile[:, 1:6:2, lo:hi],
                         sx_tile[:, 0:5:2, lo + 1 : hi + 1]),
                    ]
                if xhi > xlo:
                    pairs.append(
                        (sx_tile[:, 6, xlo:xhi],
                         sx_tile[:, 4, xlo + WP : xhi + WP]))
                for o, i in pairs:
                    if eng == "dma":
                        nc.sync.dma_start(out=o, in_=i)
                    elif eng == "gpsimd":
                        nc.gpsimd.dma_start(out=o, in_=i)
                    elif eng == "vector":
                        nc.vector.tensor_copy(o, i)
                    else:
                        nc.gpsimd.tensor_copy(out=o, in_=i)

            def plane_dst(sx_tile, cc):
                if mode == "bf16":
                    return sx_tile[:, cc, :].rearrange("p (h w) -> p h w",
                                                       w=WP)
                return sx_tile[:, 2 * cc, :PADPIX].rearrange(
                    "p (h w) -> p h w", w=WP)

            def load_sign_img0(sx_tile):
                """Lead-in image: two half-image DMAs per cc (6 big DMAs off
                the HWDGE FIFO), signs per half, banded shift copies on the
                gpsimd SWDGE ring. Conv chunk sets: {0..2} after part a,
                {3..6} after part b."""
                XSPLIT = 32  # x rows [0, 32) then [32, 56)
                for part, (r0, r1) in enumerate(((0, XSPLIT), (XSPLIT, H))):
                    for cc in range(NCC):
                        xin = xin0pool.tile([128, XSPLIT, W], F32,
                                            name="xin0", tag="xin0")
                        nc.sync.dma_start(
                            out=xin[:, : r1 - r0, :],
                            in_=x_d[0, cc * 128 : (cc + 1) * 128, r0:r1],
                        )
                        dst = plane_dst(sx_tile, cc)
                        nc.scalar.activation(
                            dst[:, 1 + r0 : 1 + r1, 1 : 1 + W],
                            xin[:, : r1 - r0, :],
                            mybir.ActivationFunctionType.Sign,
                        )
                    if mode == "fp8":
                        # B/X2 rows [0,33) serve conv chunks 0-2; rows
                        # [32,58) the rest. SWDGE keeps these off the HWDGE
                        # FIFO so part b's loads are not blocked behind them.
                        make_shift_copies(
                            sx_tile,
                            band=(0, 33) if part == 0 else (32, HP),
                            engine="gpsimd",
                        )

            def load_sign_img(img, sx_tile):
                """DMA x chunks and write sign into A planes (cc-major)."""
                for cc in range(NCC):
                    dst = plane_dst(sx_tile, cc)
                    for chunk in range(NCHUNK):
                        y0 = chunk * CHUNK_ROWS
                        xin = xinpool.tile([128, CHUNK_ROWS, W], F32,
                                           name="xin", tag="xin")
                        nc.sync.dma_start(
                            out=xin,
                            in_=x_d[img, cc * 128 : (cc + 1) * 128,
                                    y0 : y0 + CHUNK_ROWS],
                        )
                        nc.scalar.activation(
                            dst[:, 1 + y0 : 1 + y0 + CHUNK_ROWS, 1 : 1 + W],
                            xin, mybir.ActivationFunctionType.Sign,
                        )

            loop_cm = (tc.For_i(0, timing_iters, 1) if timing_iters
                       else contextlib.nullcontext())
            with loop_cm:
                # ---- pass A: conv1, stats, s1 resident in fp16 ----
                s1 = {}
                s2 = {}
                cout1 = {}
                for img in range(bpc):
                    sx_tile = sxt[img % 2]
                    early = EARLY_IMG0 and img == 0 and mode == "fp8"
                    if early:
                        load_sign_img0(sx_tile)
                    else:
                        load_sign_img(img, sx_tile)
                        if mode == "fp8":
                            make_shift_copies(sx_tile)
                    if mode == "fp8" and img == 1:
                        # w2 weight loads: overlap conv1, after img0+img1
                        # loads in the HWDGE FIFO
                        nc.sync.dma_start(out=w2p_sb, in_=w2p_d)
                        nc.sync.dma_start(out=w2s_sb, in_=w2s_d)
                    for pc in range(NPC):
                        s1t = accpool.tile([128, NPIX], F16,
                                           name=f"s1_{img}_{pc}", tag="acc")
                        s1[(img, pc)] = s1t
                        if mode == "bf16":
                            for chunk in range(NCHUNK):
                                ps = _emit_conv_bf16(nc, psum_pool, wsb1,
                                                     sx_tile, pc, chunk)
                                sl = slice(chunk * CHW, (chunk + 1) * CHW)
                                nc.scalar.copy(s1t[:, sl], ps[:])
                                nc.vector.bn_stats(
                                    out=bnst1[pc][:, img * NCHUNK + chunk, :],
                                    in_=ps[:],
                                )
                        else:
                            # img0: start matmuls before the whole image is
                            # loaded; last group: overlap evacuations with
                            # matmuls so the bn1 stats chain issues early
                            csets = None
                            if early:
                                csets = [range(0, 3), range(3, NCHUNK)]
                            elif img == bpc - 1 and pc == NPC - 1:
                                csets = [range(0, 4), range(4, NCHUNK)]
                            pss = _emit_conv_fp8(nc, psum_pool, w1p_v, w1s_v,
                                                 sx_tile, pc, chunk_sets=csets)
                            for chunk in range(NCHUNK):
                                ps_v = (pss[chunk]
                                        .rearrange("p (r c) -> p r c", c=WP)
                                        [:, :, 0:W])
                                sl = slice(chunk * CHW, (chunk + 1) * CHW)
                                s1_v = s1t[:, sl].rearrange(
                                    "p (r c) -> p r c", c=W)
                                nc.scalar.copy(s1_v, ps_v)
                                nc.vector.bn_stats(
                                    out=bnst1[pc][:, img * NCHUNK + chunk, :],
                                    in_=s1t[:, sl],
                                )
                        if img == bpc - 1:
                            for gi, (p0, p1) in enumerate(AR_GROUPS1):
                                if pc == p1 - 1:
                                    cout1[gi] = emit_group_stats(
                                        bnst1, list(range(p0, p1)),
                                        f"bn1{gi}")

                # ---- bn1: finish groups -> per-pc scale/bias views ----
                a1v, c1v = [None] * NPC, [None] * NPC
                for gi, (p0, p1) in enumerate(AR_GROUPS1):
                    a, c = finish_group_stats(*cout1[gi], 0, 1, f"bn1{gi}",
                                              p0)
                    for k, pc in enumerate(range(p0, p1)):
                        a1v[pc] = a[:, k : k + 1, :]
                        c1v[pc] = c[:, k : k + 1, :]

                # ---- W2 into the shared weight slots (bf16 mode) ----
                if mode == "bf16":
                    wsb2 = []
                    for cc in range(NCC):
                        t2 = wpool.tile([128, 9 * NPC * 128], BF16,
                                        name=f"w2sb{cc}", tag="w")
                        nc.sync.dma_start(out=t2, in_=w2_d[cc])
                        wsb2.append(t2)

                # ---- pass B: sign threshold, conv2 + residual, stats ----
                cout2 = {}
                for img in range(bpc):
                    sh_tile = sxt[img % 2]
                    # img0's signs gate the post-AllReduce restart: split
                    # top/bottom so conv2 chunks 0-2 start after the top
                    # rows are thresholded instead of the whole image
                    parts = (((0, 32), (32, H))
                             if img == 0 and EARLY_IMG0 and mode == "fp8"
                             else ((0, H),))
                    for part, (r0, r1) in enumerate(parts):
                        for pc in range(NPC):
                            srcv = s1[(img, pc)].rearrange(
                                "p (h w) -> p h w", w=W)
                            dst = plane_dst(sh_tile, pc)
                            nc.scalar.activation(
                                dst[:, 1 + r0 : 1 + r1, 1 : 1 + W],
                                srcv[:, r0:r1, :],
                                mybir.ActivationFunctionType.Sign,
                                bias=c1v[pc][:, 0, :], scale=a1v[pc][:, 0, :],
                            )
                        if mode == "fp8":
                            if len(parts) == 1:
                                make_shift_copies(sh_tile)
                            else:
                                make_shift_copies(
                                    sh_tile,
                                    band=(0, 33) if part == 0 else (32, HP),
                                )
                    for pc in range(NPC):
                        s2t = accpool.tile([128, NPIX], F16,
                                           name=f"s2_{img}_{pc}", tag="acc")
                        s2[(img, pc)] = s2t
                        if mode == "bf16":
                            pss = [_emit_conv_bf16(nc, psum_pool, wsb2,
                                                   sh_tile, pc, chunk)
                                   for chunk in range(NCHUNK)]
                        else:
                            # the barrier-gating last groups (pc1 gates the
                            # group-A AllReduce, pc2 group-B) get two chunk
                            # sets so their evacuations overlap their matmuls
                            # (stop-matmuls otherwise all land at group end)
                            csets = ([range(0, 4), range(4, NCHUNK)]
                                     if img == bpc - 1 and pc >= 1 else None)
                            pss = _emit_conv_fp8(nc, psum_pool, w2p_v, w2s_v,
                                                 sh_tile, pc,
                                                 chunk_sets=csets)
                        for chunk in range(NCHUNK):
                            y0 = chunk * CHUNK_ROWS
                            xr = xrpool.tile([128, CHUNK_ROWS, W], F32,
                                             name="xr", tag="xr")
                            nc.sync.dma_start(
                                out=xr,
                                in_=x_d[img, pc * 128 : (pc + 1) * 128,
                                        y0 : y0 + CHUNK_ROWS],
                            )
                            sl = slice(chunk * CHW, (chunk + 1) * CHW)
                            if mode == "bf16":
                                ps_in = pss[chunk][:]
                                xr_in = xr.rearrange("p h w -> p (h w)")
                                out_ap = s2t[:, sl]
                                nc.vector.tensor_tensor(
                                    out=out_ap, in0=ps_in, in1=xr_in,
                                    op=mybir.AluOpType.add,
                                )
                            else:
                                # evacuation spread over three engines:
                                # scalar strips the seam out of PSUM, gpsimd
                                # adds the residual (SBUF-only engine), DVE
                                # just does bn_stats -- keeps the DVE tail
                                # off the stats chain's critical path
                                ps_in = (pss[chunk]
                                         .rearrange("p (r c) -> p r c", c=WP)
                                         [:, :, 0:W])
                                out_ap = s2t[:, sl].rearrange(
                                    "p (r c) -> p r c", c=W)
                                nc.scalar.copy(out_ap, ps_in)
                                nc.gpsimd.tensor_tensor(
                                    out=out_ap, in0=out_ap, in1=xr[:],
                                    op=mybir.AluOpType.add,
                                )
                            nc.vector.bn_stats(
                                out=bnst2[pc][:, img * NCHUNK + chunk, :],
                                in_=s2t[:, sl],
                            )
                        if img == bpc - 1:
                            for gi, (p0, p1) in enumerate(AR_GROUPS2):
                                if pc == p1 - 1:
                                    cout2[gi] = emit_group_stats(
                                        bnst2, list(range(p0, p1)),
                                        f"bn2{gi}")

                # ---- bn2 finish + pass C, AR-group at a time: group A's
                # scale/bias lands before the group-B AllReduce completes, so
                # 2/3 of the output DMA overlaps it. Emitting finish(B) after
                # group A's stores also keeps its DRAM readback from
                # head-of-line-blocking them in the HWDGE FIFO. ----
                a2v, c2v = [None] * NPC, [None] * NPC

                def emit_pass_c(img, pc):
                    s2t = s2[(img, pc)]
                    for chunk in range(NCHUNK):
                        sl = slice(chunk * CHW, (chunk + 1) * CHW)
                        oc = ocpool.tile([128, CHW], F32, name="oc",
                                         tag="oc")
                        ocs = oc[:]
                        if chunk % 2 == 0:
                            nc.scalar.activation(
                                ocs, s2t[:, sl],
                                mybir.ActivationFunctionType.Identity,
                                bias=c2v[pc][:, 0, :], scale=a2v[pc][:, 0, :],
                            )
                            nc.vector.tensor_scalar(
                                out=ocs, in0=ocs, scalar1=1.0,
                                scalar2=-1.0, op0=mybir.AluOpType.min,
                                op1=mybir.AluOpType.max,
                            )
                        else:
                            nc.vector.tensor_scalar(
                                out=ocs, in0=s2t[:, sl],
                                scalar1=a2v[pc][:, 0, :],
                                scalar2=c2v[pc][:, 0, :],
                                op0=mybir.AluOpType.mult,
                                op1=mybir.AluOpType.add,
                            )
                            nc.gpsimd.tensor_scalar(
                                out=ocs, in0=ocs, scalar1=1.0,
                                scalar2=-1.0, op0=mybir.AluOpType.min,
                                op1=mybir.AluOpType.max,
                            )
                        y0 = chunk * CHUNK_ROWS
                        nc.sync.dma_start(
                            out=out_d[img, pc * 128 : (pc + 1) * 128,
                                      y0 : y0 + CHUNK_ROWS],
                            in_=oc.rearrange("p (h w) -> p h w", w=W),
                        )

                for gi, (p0, p1) in enumerate(AR_GROUPS2):
                    a, c = finish_group_stats(*cout2[gi], 2, 3, f"bn2{gi}",
                                              p0)
                    for k, pc in enumerate(range(p0, p1)):
                        a2v[pc] = a[:, k : k + 1, :]
                        c2v[pc] = c[:, k : k + 1, :]
                    for img in range(bpc):
                        for pc in range(p0, p1):
                            emit_pass_c(img, pc)

    nc.compile()
    return nc


_PROGRAM = None


def _get_program():
    global _PROGRAM
    if _PROGRAM is None:
        _PROGRAM = build_program()
    return _PROGRAM


def make_in_maps(x, W1, W2, g1, b1, g2, b2, bpc=BPC, ncores=NCORES,
                 mode=MODE):
    vecs = _prep_vecs(np.asarray(g1), np.asarray(b1), np.asarray(g2),
                      np.asarray(b2))
    x = np.ascontiguousarray(np.asarray(x, dtype=np.float32))
    if mode == "bf16":
        wmap = {"w1": _prep_weight(np.asarray(W1)),
                "w2": _prep_weight(np.asarray(W2))}
    else:
        w1p, w1s = _prep_weight_fp8(np.asarray(W1))
        w2p, w2s = _prep_weight_fp8(np.asarray(W2))
        wmap = {"w1p": w1p, "w1s": w1s, "w2p": w2p, "w2s": w2s}
    return [
        {"x": x[core * bpc : (core + 1) * bpc], "vecs": vecs, **wmap}
        for core in range(ncores)
    ]


def kernel(x, W1, W2, g1, b1, g2, b2, trace=False):
    nc = _get_program()
    in_maps = make_in_maps(x, W1, W2, g1, b1, g2, b2)
    res = run_bass_kernel_spmd(
        nc, in_maps, core_ids=list(range(NCORES)), trace=trace
    )
    out = np.concatenate([res.results[c]["out"] for c in range(NCORES)], axis=0)
    kernel.last_results = res
    return out


# revision 29
# speedup vs baseline: 1.0616x; 1.0616x over previous
"""Binarized ResNet BasicBlock (2x binarized 3x3 conv + batchnorm + hardtanh,
residual) on 8 Trainium2 NeuronCores, data-parallel over batch.

Math (per reference):
  s1  = conv3x3(sign(x), sign(W1), pad=1)          # integer-valued
  h   = clip(bn1(s1), -1, 1)                       # only sign(h) is consumed
  s2p = conv3x3(sign(h), sign(W2), pad=1) + x
  out = clip(bn2(s2p), -1, 1)

Key points:
  - sign(h) = sign(a1*s1 + c1) per channel (a1 = g1*rsqrt(v1+eps),
    c1 = b1 - m1*a1), so h is never materialized.
  - batchnorm needs global batch stats: each core computes per-channel
    (E[x], E[x^2]) partials over its 4 images; tiny AllReduces combine them
    (equal pixel counts per core, so mean-of-means works). Each barrier's
    AllReduce is split in two: {pc0, pc1} issued right after their last conv
    group (hidden under the remaining pc2 conv work), {pc2} at the end (the
    only exposed latency).
  - fp8 mode (default): +/-1 activations/weights in fp8e4 are exact; the
    3x3 conv's 27 (channel-chunk, tap) units are packed into 13 DoubleRow
    K=256 matmuls + 1 normal K=128 matmul per output tile. The rhs must be
    a depth-2 AP (pair dim + one run), so each tile reads contiguous
    464-wide runs of the 58-col padded plane and the evacuation strips the
    2-col seam. Pair base addresses must be 2B-aligned and pair strides
    16B-aligned, so shifted copies of the sign plane provide the dx=1 taps
    (shift +1 col) and a row-shifted plane pairs up the leftover dx=2 taps.
  - s1 and s2p stay resident in SBUF as fp16 (integers < 2048: exact; s2p
    adds the fp32 residual, fp16 rounding ~5e-4 relative).
  - lead-in: image 0 is loaded chunk-major with banded shift copies and its
    conv1 split in two chunk-sets, so the first matmul starts after ~2/7 of
    the image instead of the whole image. Plane buffers get border-only
    memsets (the shift copies cover B/X2 planes entirely).
  - pass C writes pc0/pc1 outputs first: their scale/bias comes from the
    early AllReduce, so the output DMA starts at conv2 end, overlapping the
    exposed pc2 AllReduce.
"""

import contextlib

import numpy as np
import ml_dtypes

import concourse.bass as bass
import concourse.tile as tile
from concourse import bacc, mybir
from concourse.bass_utils import run_bass_kernel_spmd
from concourse.replica_groups import maybe_share_collective_output_space

F32 = mybir.dt.float32
F16 = mybir.dt.float16
BF16 = mybir.dt.bfloat16
F8 = mybir.dt.float8e4
F8NP = mybir.dt.np(F8)

NCORES = 8
B, C, H, W = 32, 384, 56, 56
P = C
BPC = B // NCORES         # images per core
NCC = C // 128            # input channel chunks
NPC = P // 128            # output channel chunks
HP = H + 2                # padded rows
WP = W + 2                # padded cols
PADPIX = HP * WP          # 3364
NPIX = H * W              # 3136
CHUNK_ROWS = 8            # output rows per PSUM tile
NCHUNK = H // CHUNK_ROWS  # 7
CHW = CHUNK_ROWS * W      # 448
EPS = 1e-5

MODE = "fp8"              # "bf16" or "fp8"
FP8_SWIL = False          # plain DoubleRow beat SwInterleave on HW
SHIFT_ENGINE = "dma"      # "dma" | "gpsimd" | "vector": shifted-plane copies
FINE_COPIES = True        # per-plane shift copies (finer deps; A/B: -25us)
DEEP_BUFS = True          # xin/xr/oc staging depth 8 instead of 6
SPLIT_AR = True           # split each stats AllReduce into {pc0,pc1} + {pc2}
EARLY_IMG0 = True         # chunk-major img0 load + banded copies + 2 chunk sets
CSTRIDE = 3376            # fp8 padded plane stride (16B-aligned)
RUN = CHUNK_ROWS * WP     # 464 <= 512: contiguous rhs run incl. seam
NPLANE = 7                # A0 B0 A1 B1 A2 B2 X2

# fp8 unit schedule: 13 DoubleRow pairs + 1 single cover the 27 (cc, dy, dx)
# conv units. Planes: A-cc at 2cc (shift 0), B-cc at 2cc+1 (shift +1 col,
# provides dx=1 taps at even base addresses), X2 at 6 (A2 shifted one row,
# provides the dy+1 partner for cc2 dx=2 taps).
#  dx01 pair (cc, dy): taps (cc,dy,0)@A-cc, (cc,dy,1)@B-cc;
#    rhs sx[:, 2cc:2cc+2, q:q+RUN], q=(y0+dy)*WP
#  cc01 pair (dy): taps (0,dy,2)@A0, (1,dy,2)@A1;
#    rhs sx[:, 0:3:2, q:q+RUN], q=(y0+dy)*WP+2
#  xp pair: taps (2,0,2)@A2, (2,1,2)@X2; rhs sx[:, 4:7:2, q:q+RUN],
#    q=y0*WP+2
#  single: tap (2,2,2)@A2; rhs sx[:, 4, q:q+RUN], q=(y0+2)*WP+2
FP8_PAIRS = (
    [("dx01", cc, dy) for dy in range(3) for cc in range(3)]
    + [("cc01", None, dy) for dy in range(3)]
    + [("xp", None, None)]
)
NUNIT_FP8 = len(FP8_PAIRS) + 1  # 14

# AllReduce pc grouping. bn1: one AR (conv2 needs every pc's threshold, so
# splitting only serializes collectives for nothing). bn2: group A = pc 0..1
# issued one conv group early (its scale/bias lands by conv2 end, so 2/3 of
# the output store overlaps the exposed group-B AllReduce).
AR_GROUPS1 = [(0, NPC)]
AR_GROUPS2 = [(0, 2), (2, 3)] if SPLIT_AR else [(0, NPC)]


def _prep_weight(w):
    """bf16 mode: [P, C, 3, 3] -> [NCC, 128, 9*NPC*128] bf16 sign; lhsT for
    (cc, off, pc) is wsb[cc][:, (off*NPC+pc)*128 : +128]."""
    ws = np.sign(w.astype(np.float32)).astype(ml_dtypes.bfloat16)
    arr = ws.transpose(1, 2, 3, 0).reshape(C, 9, P)
    arr = arr.reshape(NCC, 128, 9, NPC, 128)
    return np.ascontiguousarray(arr.reshape(NCC, 128, 9 * NPC * 128))


def _fp8_pair_units():
    """(uA, uB) tap indices per FP8_PAIRS entry; each tap is (cc, dy, dx)."""
    out = []
    for kind, cc, dy in FP8_PAIRS:
        if kind == "dx01":
            out.append(((cc, dy, 0), (cc, dy, 1)))
        elif kind == "cc01":
            out.append(((0, dy, 2), (1, dy, 2)))
        else:
            out.append(((2, 0, 2), (2, 1, 2)))
    return out


def _prep_weight_fp8(w):
    """[P, C, 3, 3] -> (pairs [128, 13*NPC*256], single [128, NPC*128]) fp8
    sign values. SwInterleave layout: per pair/pc block of 256, columns are
    [A127, B127, A126, B126, ..., A0, B0] (interleaved, reversed)."""
    ws = np.sign(w.astype(np.float32))
    arr = ws.transpose(1, 2, 3, 0).reshape(NCC, 128, 3, 3, NPC, 128)

    def unit(cc, dy, dx):  # [128 (c), NPC, 128 (m)]
        return arr[cc, :, dy, dx]

    npair = len(FP8_PAIRS)
    if FP8_SWIL:
        wp = np.zeros((128, npair, NPC, 256), np.float32)
        for j, (uA, uB) in enumerate(_fp8_pair_units()):
            wp[:, j, :, 0::2] = unit(*uA)[:, :, ::-1]
            wp[:, j, :, 1::2] = unit(*uB)[:, :, ::-1]
    else:
        wp = np.zeros((128, npair, NPC, 2, 128), np.float32)
        for j, (uA, uB) in enumerate(_fp8_pair_units()):
            wp[:, j, :, 0] = unit(*uA)
            wp[:, j, :, 1] = unit(*uB)
    wsg = unit(2, 2, 2)  # [128, NPC, 128]
    return (
        np.ascontiguousarray(wp.reshape(128, -1)).astype(F8NP),
        np.ascontiguousarray(wsg.reshape(128, -1)).astype(F8NP),
    )


def _prep_vecs(g1, b1, g2, b2):
    """-> [128, NPC, 4] f32: per-partition (p_in) per-chunk (pc) gamma/beta."""
    out = np.empty((128, NPC, 4), np.float32)
    for k, v in enumerate((g1, b1, g2, b2)):
        out[:, :, k] = v.astype(np.float32).reshape(NPC, 128).T
    return out


def _stats_to_scale_bias(nc, singles, allout, vecs_sb, eps_tile, gk, bk, name,
                         ncores, width):
    """allout [128, width, 2] summed (E, E2) over cores -> a, c [128, width, 1].
    vecs_sb is the matching [128, width, 4] slice."""
    g = width
    Eg = singles.tile([128, g, 1], F32, name=f"{name}_Eg")
    E2g = singles.tile([128, g, 1], F32, name=f"{name}_E2g")
    var = singles.tile([128, g, 1], F32, name=f"{name}_var")
    tmp = singles.tile([128, g, 1], F32, name=f"{name}_tmp")
    sd = singles.tile([128, g, 1], F32, name=f"{name}_sd")
    rs = singles.tile([128, g, 1], F32, name=f"{name}_rs")
    a = singles.tile([128, g, 1], F32, name=f"{name}_a")
    c = singles.tile([128, g, 1], F32, name=f"{name}_c")
    nc.scalar.mul(Eg[:], allout[:, :, 0:1], 1.0 / ncores)
    nc.scalar.mul(E2g[:], allout[:, :, 1:2], 1.0 / ncores)
    nc.vector.tensor_mul(tmp[:], Eg[:], Eg[:])
    nc.vector.tensor_tensor(
        out=var[:], in0=E2g[:], in1=tmp[:], op=mybir.AluOpType.subtract
    )
    nc.scalar.activation(
        sd[:], var[:], mybir.ActivationFunctionType.Sqrt, bias=eps_tile[:],
        scale=1.0,
    )
    nc.vector.reciprocal(out=rs[:], in_=sd[:])
    nc.vector.tensor_mul(a[:], rs[:], vecs_sb[:, :, gk : gk + 1])
    nc.vector.tensor_mul(tmp[:], Eg[:], a[:])
    nc.vector.tensor_tensor(
        out=c[:], in0=vecs_sb[:, :, bk : bk + 1], in1=tmp[:],
        op=mybir.AluOpType.subtract,
    )
    return a, c


def _emit_conv_bf16(nc, psum_pool, wsb, sx_tile, pc, chunk):
    """27 accumulated bf16 matmuls -> psum tile [128, CHW]."""
    ps = psum_pool.tile([128, CHW], F32, name="ps", tag="ps")
    y0 = chunk * CHUNK_ROWS
    k = 0
    for cc in range(NCC):
        sx3 = sx_tile[:, cc, :].rearrange("p (h w) -> p h w", w=WP)
        for off in range(9):
            dy, dx = off // 3, off % 3
            lhsT = wsb[cc][:, (off * NPC + pc) * 128 : (off * NPC + pc + 1) * 128]
            rhs = sx3[:, y0 + dy : y0 + dy + CHUNK_ROWS, dx : dx + W]
            nc.tensor.matmul(
                ps[:], lhsT, rhs, start=(k == 0), stop=(k == 9 * NCC - 1)
            )
            k += 1
    return ps


def _emit_conv_fp8(nc, psum_pool, wp_view, ws_view, sx_tile, pc,
                   chunk_sets=None):
    """Weight-stationary fp8 DoubleRow conv for one (img, pc): returns NCHUNK
    psum tiles [128, RUN]; valid output cols = (8, 58) view sliced [:, :56].
    By default all 7 chunks accumulate in one weight-stationary pass (7 of 8
    PSUM banks; splitting into half-groups measured slower for steady-state
    images -- the extra DoubleRow LDWEIGHTS cost more than the bank stalls
    they saved). chunk_sets overrides the grouping (used for image 0's
    lead-in, where starting on the first rows early wins)."""
    perf = (mybir.MatmulPerfMode.DoubleRowSwInterleave if FP8_SWIL
            else mybir.MatmulPerfMode.DoubleRow)
    if chunk_sets is None:
        chunk_sets = [range(0, NCHUNK)]
    pss = {}
    for cset in chunk_sets:
        for chunk in cset:
            pss[chunk] = psum_pool.tile([128, RUN], F32, name="ps", tag="ps")
        u = 0
        for j, (kind, cc, dy) in enumerate(FP8_PAIRS):
            lhsT = wp_view[:, j, pc]
            for chunk in cset:
                y0 = chunk * CHUNK_ROWS
                if kind == "dx01":
                    q = (y0 + dy) * WP
                    rhs = sx_tile[:, 2 * cc : 2 * cc + 2, q : q + RUN]
                elif kind == "cc01":
                    q = (y0 + dy) * WP + 2
                    rhs = sx_tile[:, 0:3:2, q : q + RUN]
                else:  # xp
                    q = y0 * WP + 2
                    rhs = sx_tile[:, 4:7:2, q : q + RUN]
                nc.tensor.matmul(
                    pss[chunk][:], lhsT, rhs,
                    start=(u == 0), stop=(u == NUNIT_FP8 - 1), perf_mode=perf,
                )
            u += 1
        lhsT = ws_view[:, pc]
        for chunk in cset:
            y0 = chunk * CHUNK_ROWS
            q = (y0 + 2) * WP + 2
            rhs = sx_tile[:, 4, q : q + RUN]
            nc.tensor.matmul(
                pss[chunk][:], lhsT, rhs,
                start=(u == 0), stop=(u == NUNIT_FP8 - 1),
            )
    return [pss[c] for c in range(NCHUNK)]


def build_program(bpc=BPC, ncores=NCORES, mode=MODE, timing_iters=None):
    nc = bacc.Bacc(
        "TRN2",
        target_bir_lowering=False,
        debug=False,
        enable_asserts=True,
        num_devices=ncores,
    )
    x_d = nc.dram_tensor("x", [bpc, C, H, W], F32, kind="ExternalInput").ap()
    if mode == "bf16":
        w1_d = nc.dram_tensor("w1", [NCC, 128, 9 * NPC * 128], BF16,
                              kind="ExternalInput").ap()
        w2_d = nc.dram_tensor("w2", [NCC, 128, 9 * NPC * 128], BF16,
                              kind="ExternalInput").ap()
    else:
        wpair_elems = len(FP8_PAIRS) * NPC * 256
        w1p_d = nc.dram_tensor("w1p", [128, wpair_elems], F8,
                               kind="ExternalInput").ap()
        w1s_d = nc.dram_tensor("w1s", [128, NPC * 128], F8,
                               kind="ExternalInput").ap()
        w2p_d = nc.dram_tensor("w2p", [128, wpair_elems], F8,
                               kind="ExternalInput").ap()
        w2s_d = nc.dram_tensor("w2s", [128, NPC * 128], F8,
                               kind="ExternalInput").ap()
    vecs_d = nc.dram_tensor("vecs", [128, NPC, 4], F32,
                            kind="ExternalInput").ap()
    out_d = nc.dram_tensor("out", [bpc, C, H, W], F32,
                           kind="ExternalOutput").ap()

    with tile.TileContext(nc) as tc:
        with (
            tc.tile_pool(name="weights",
                         bufs=NCC if mode == "bf16" else 2) as wpool,
            tc.tile_pool(name="singles", bufs=1) as singles,
            tc.tile_pool(name="sx", bufs=1) as sxpool,
            tc.tile_pool(name="acc", bufs=3 * bpc) as accpool,
            tc.tile_pool(name="xin", bufs=8 if DEEP_BUFS else 6) as xinpool,
            tc.tile_pool(name="xin0", bufs=2) as xin0pool,
            tc.tile_pool(name="xr", bufs=8 if DEEP_BUFS else 6) as xrpool,
            tc.tile_pool(name="oc", bufs=8 if DEEP_BUFS else 6) as ocpool,
            tc.tile_pool(name="stats", bufs=1) as stpool,
            tc.tile_pool(name="psum", bufs=8, space="PSUM") as psum_pool,
            tc.tile_pool(name="dram", bufs=1, space="DRAM") as dram,
        ):
            # ---- constants (outside the timing loop) ----
            if mode == "bf16":
                # W1/W2 share slots (tag "w"): W2 allocates after conv1
                # releases W1, overlapping the bn1 AllReduce.
                wsb1 = []
                for cc in range(NCC):
                    t1 = wpool.tile([128, 9 * NPC * 128], BF16,
                                    name=f"w1sb{cc}", tag="w")
                    nc.sync.dma_start(out=t1, in_=w1_d[cc])
                    wsb1.append(t1)
            else:
                w1p_sb = wpool.tile([128, len(FP8_PAIRS) * NPC * 256], F8,
                                    name="w1p_sb", tag="wp")
                nc.sync.dma_start(out=w1p_sb, in_=w1p_d)
                w1s_sb = wpool.tile([128, NPC * 128], F8, name="w1s_sb",
                                    tag="ws")
                nc.sync.dma_start(out=w1s_sb, in_=w1s_d)
                # w2p/w2s load on the gpsimd SWDGE ring: off the sync HWDGE
                # FIFO so they do not sit ahead of image 0's loads there.
                w2p_sb = wpool.tile([128, len(FP8_PAIRS) * NPC * 256], F8,
                                    name="w2p_sb", tag="wp")
                nc.gpsimd.dma_start(out=w2p_sb, in_=w2p_d)
                w2s_sb = wpool.tile([128, NPC * 128], F8, name="w2s_sb",
                                    tag="ws")
                nc.gpsimd.dma_start(out=w2s_sb, in_=w2s_d)
                if FP8_SWIL:
                    w1p_v = w1p_sb.rearrange("p (j q m) -> p j q m",
                                             j=len(FP8_PAIRS), q=NPC)
                    w2p_v = w2p_sb.rearrange("p (j q m) -> p j q m",
                                             j=len(FP8_PAIRS), q=NPC)
                else:
                    w1p_v = w1p_sb.rearrange("p (j q i m) -> p j q i m",
                                             j=len(FP8_PAIRS), q=NPC, i=2)
                    w2p_v = w2p_sb.rearrange("p (j q i m) -> p j q i m",
                                             j=len(FP8_PAIRS), q=NPC, i=2)
                w1s_v = w1s_sb.rearrange("p (q m) -> p q m", q=NPC)
                w2s_v = w2s_sb.rearrange("p (q m) -> p q m", q=NPC)
            vecs_sb = singles.tile([128, NPC, 4], F32)
            nc.sync.dma_start(out=vecs_sb, in_=vecs_d)
            eps_tile = singles.tile([128, 1], F32)
            nc.vector.memset(eps_tile, EPS)

            # persistent padded sign planes (borders stay zero forever)
            sxt = []
            for s in range(2):
                if mode == "bf16":
                    t = sxpool.tile([128, NCC, PADPIX], BF16, name=f"sx{s}")
                    nc.gpsimd.memset(t[:], 0.0)
                else:
                    t = sxpool.tile([128, NPLANE, CSTRIDE], F8, name=f"sx{s}")
                    # Only A planes need zero borders: the sign writes cover
                    # rows 1..56 cols 1..56; shift copies fill B/X2 planes
                    # entirely from A (plus A's borders). Memset only the
                    # border strips, split across idle engines.
                    for icc in range(NCC):
                        pl = 2 * icc
                        eng = (nc.vector, nc.gpsimd)[icc % 2]
                        pv = t[:, pl, :]
                        eng.memset(pv[0:128, 0:WP], 0.0)
                        eng.memset(pv[0:128, (HP - 1) * WP : CSTRIDE], 0.0)
                        # cols 0 and 57 for rows 1..56, one strided AP
                        colv = t[:, pl, WP : WP + (H * WP)].rearrange(
                            "p (h w) -> p h w", w=WP)
                        eng.memset(colv[:, :, 0 : WP : WP - 1], 0.0)
                sxt.append(t)

            bnst1 = [
                stpool.tile([128, bpc * NCHUNK, 6], F32, name=f"bnst1_{pc}")
                for pc in range(NPC)
            ]
            bnst2 = [
                stpool.tile([128, bpc * NCHUNK, 6], F32, name=f"bnst2_{pc}")
                for pc in range(NPC)
            ]

            cc_addr_space = (
                "Local" if timing_iters is not None
                else maybe_share_collective_output_space(
                    "AllReduce", [list(range(ncores))]
                )
            )

            def do_allreduce(cin, cout):
                if timing_iters is None:
                    nc.gpsimd.collective_compute(
                        "AllReduce",
                        mybir.AluOpType.add,
                        replica_groups=[list(range(ncores))],
                        ins=[cin.opt()],
                        outs=[cout.opt()],
                    )
                else:
                    nc.gpsimd.dma_start(out=cout, in_=cin)

            def emit_group_stats(bnst, pcs, tag):
                """bn_aggr + (E, E2) combine for pcs -> DRAM -> AllReduce.
                Returns (cout_dram, width)."""
                g = len(pcs)
                allin = singles.tile([128, g, 2], F32, name=f"allin_{tag}")
                for k, pc in enumerate(pcs):
                    mv = stpool.tile([128, 2], F32, name=f"mv_{tag}_{pc}")
                    nc.vector.bn_aggr(out=mv, in_=bnst[pc])
                    nc.vector.tensor_copy(allin[:, k, 0:1], mv[:, 0:1])
                    sq = stpool.tile([128, 1], F32, name=f"sq_{tag}_{pc}")
                    nc.vector.tensor_mul(sq, mv[:, 0:1], mv[:, 0:1])
                    nc.vector.tensor_tensor(
                        out=allin[:, k, 1:2], in0=mv[:, 1:2], in1=sq,
                        op=mybir.AluOpType.add,
                    )
                cin = dram.tile([128, g * 2], F32, name=f"cc_{tag}_in")
                cout = dram.tile([128, g * 2], F32, name=f"cc_{tag}_out",
                                 addr_space=cc_addr_space)
                nc.sync.dma_start(
                    out=cin, in_=allin.rearrange("p a b -> p (a b)"))
                do_allreduce(cin, cout)
                return cout, g

            def finish_group_stats(cout, g, gk, bk, tag, pc0):
                """DRAM result -> SBUF -> per-channel scale/bias [128, g, 1]."""
                allout = singles.tile([128, g, 2], F32, name=f"allout_{tag}")
                # sync ring (NOT gpsimd: a readback there would sit between
                # the two collectives in the gpsimd FIFO and serialize them).
                # Emission order keeps anything urgent from queuing behind it.
                nc.sync.dma_start(
                    out=allout.rearrange("p a b -> p (a b)"), in_=cout)
                vs = vecs_sb[:, pc0 : pc0 + g, :]
                return _stats_to_scale_bias(
                    nc, singles, allout, vs, eps_tile, gk, bk, tag, ncores, g)

            def make_shift_copies(sx_tile, band=None, engine=None):
                """B planes = A planes shifted +1 col; X2 = A2 shifted one
                row. Pad regions supply the zeros. band=(r0, r1) restricts
                the copy to plane rows [r0, r1) (X2 gets [r0, r1-1) capped)."""
                eng = engine or SHIFT_ENGINE
                if band is None:
                    b0, b1 = 0, CSTRIDE // WP  # rows 0..58
                    lo, hi = 0, CSTRIDE - 1
                    xlo, xhi = 0, CSTRIDE - WP
                else:
                    b0, b1 = band
                    lo, hi = b0 * WP, min(b1 * WP, CSTRIDE - 1)
                    xlo, xhi = b0 * WP, min((b1 - 1) * WP, CSTRIDE - WP)
                if FINE_COPIES:
                    pairs = [
                        (sx_tile[:, 2 * cc + 1, lo:hi],
                         sx_tile[:, 2 * cc, lo + 1 : hi + 1])
                        for cc in range(NCC)
                    ]
                else:
                    pairs = [
                        (sx_tile[:, 1:6:2, lo:hi],
                         sx_tile[:, 0:5:2, lo + 1 : hi + 1]),
                    ]
                if xhi > xlo:
                    pairs.append(
                        (sx_tile[:, 6, xlo:xhi],
                         sx_tile[:, 4, xlo + WP : xhi + WP]))
                for o, i in pairs:
                    if eng == "dma":
                        nc.sync.dma_start(out=o, in_=i)
                    elif eng == "gpsimd":
                        nc.gpsimd.dma_start(out=o, in_=i)
                    elif eng == "vector":
                        nc.vector.tensor_copy(o, i)
                    else:
                        nc.gpsimd.tensor_copy(out=o, in_=i)

            def plane_dst(sx_tile, cc):
                if mode == "bf16":
                    return sx_tile[:, cc, :].rearrange("p (h w) -> p h w",
                                                       w=WP)
                return sx_tile[:, 2 * cc, :PADPIX].rearrange(
                    "p (h w) -> p h w", w=WP)

            def load_sign_img0(sx_tile):
                """Lead-in image: two half-image DMAs per cc (6 big DMAs off
                the HWDGE FIFO), signs per half, banded shift copies on the
                gpsimd SWDGE ring. Conv chunk sets: {0..2} after part a,
                {3..6} after part b."""
                XSPLIT = 32  # x rows [0, 32) then [32, 56)
                for part, (r0, r1) in enumerate(((0, XSPLIT), (XSPLIT, H))):
                    for cc in range(NCC):
                        xin = xin0pool.tile([128, XSPLIT, W], F32,
                                            name="xin0", tag="xin0")
                        nc.sync.dma_start(
                            out=xin[:, : r1 - r0, :],
                            in_=x_d[0, cc * 128 : (cc + 1) * 128, r0:r1],
                        )
                        dst = plane_dst(sx_tile, cc)
                        nc.scalar.activation(
                            dst[:, 1 + r0 : 1 + r1, 1 : 1 + W],
                            xin[:, : r1 - r0, :],
                            mybir.ActivationFunctionType.Sign,
                        )
                    if mode == "fp8":
                        # B/X2 rows [0,33) serve conv chunks 0-2; rows
                        # [32,58) the rest. SWDGE keeps these off the HWDGE
                        # FIFO so part b's loads are not blocked behind them.
                        make_shift_copies(
                            sx_tile,
                            band=(0, 33) if part == 0 else (32, HP),
                            engine="gpsimd",
                        )

            def load_sign_img(img, sx_tile):
                """DMA x chunks and write sign into A planes (cc-major)."""
                for cc in range(NCC):
                    dst = plane_dst(sx_tile, cc)
                    for chunk in range(NCHUNK):
                        y0 = chunk * CHUNK_ROWS
                        xin = xinpool.tile([128, CHUNK_ROWS, W], F32,
                                           name="xin", tag="xin")
                        nc.sync.dma_start(
                            out=xin,
                            in_=x_d[img, cc * 128 : (cc + 1) * 128,
                                    y0 : y0 + CHUNK_ROWS],
                        )
                        nc.scalar.activation(
                            dst[:, 1 + y0 : 1 + y0 + CHUNK_ROWS, 1 : 1 + W],
                            xin, mybir.ActivationFunctionType.Sign,
                        )

            loop_cm = (tc.For_i(0, timing_iters, 1) if timing_iters
                       else contextlib.nullcontext())
            with loop_cm:
                # ---- pass A: conv1, stats, s1 resident in fp16 ----
                s1 = {}
                s2 = {}
                cout1 = {}
                for img in range(bpc):
                    sx_tile = sxt[img % 2]
                    early = EARLY_IMG0 and img == 0 and mode == "fp8"
                    if early:
                        load_sign_img0(sx_tile)
                    else:
                        load_sign_img(img, sx_tile)
                        if mode == "fp8":
                            make_shift_copies(sx_tile)
                    for pc in range(NPC):
                        s1t = accpool.tile([128, NPIX], F16,
                                           name=f"s1_{img}_{pc}", tag="acc")
                        s1[(img, pc)] = s1t
                        if mode == "bf16":
                            for chunk in range(NCHUNK):
                                ps = _emit_conv_bf16(nc, psum_pool, wsb1,
                                                     sx_tile, pc, chunk)
                                sl = slice(chunk * CHW, (chunk + 1) * CHW)
                                nc.scalar.copy(s1t[:, sl], ps[:])
                                nc.vector.bn_stats(
                                    out=bnst1[pc][:, img * NCHUNK + chunk, :],
                                    in_=ps[:],
                                )
                        else:
                            # img0: start matmuls before the whole image is
                            # loaded; last group: overlap evacuations with
                            # matmuls so the bn1 stats chain issues early
                            csets = None
                            if early:
                                csets = [range(0, 3), range(3, NCHUNK)]
                            elif img == bpc - 1 and pc == NPC - 1:
                                csets = [range(0, 4), range(4, NCHUNK)]
                            pss = _emit_conv_fp8(nc, psum_pool, w1p_v, w1s_v,
                                                 sx_tile, pc, chunk_sets=csets)
                            for chunk in range(NCHUNK):
                                ps_v = (pss[chunk]
                                        .rearrange("p (r c) -> p r c", c=WP)
                                        [:, :, 0:W])
                                sl = slice(chunk * CHW, (chunk + 1) * CHW)
                                s1_v = s1t[:, sl].rearrange(
                                    "p (r c) -> p r c", c=W)
                                nc.scalar.copy(s1_v, ps_v)
                                nc.vector.bn_stats(
                                    out=bnst1[pc][:, img * NCHUNK + chunk, :],
                                    in_=s1t[:, sl],
                                )
                        if img == bpc - 1:
                            for gi, (p0, p1) in enumerate(AR_GROUPS1):
                                if pc == p1 - 1:
                                    cout1[gi] = emit_group_stats(
                                        bnst1, list(range(p0, p1)),
                                        f"bn1{gi}")

                # ---- bn1: finish groups -> per-pc scale/bias views ----
                a1v, c1v = [None] * NPC, [None] * NPC
                for gi, (p0, p1) in enumerate(AR_GROUPS1):
                    a, c = finish_group_stats(*cout1[gi], 0, 1, f"bn1{gi}",
                                              p0)
                    for k, pc in enumerate(range(p0, p1)):
                        a1v[pc] = a[:, k : k + 1, :]
                        c1v[pc] = c[:, k : k + 1, :]

                # ---- W2 into the shared weight slots (bf16 mode) ----
                if mode == "bf16":
                    wsb2 = []
                    for cc in range(NCC):
                        t2 = wpool.tile([128, 9 * NPC * 128], BF16,
                                        name=f"w2sb{cc}", tag="w")
                        nc.sync.dma_start(out=t2, in_=w2_d[cc])
                        wsb2.append(t2)

                # ---- pass B: sign threshold, conv2 + residual, stats ----
                cout2 = {}
                for img in range(bpc):
                    sh_tile = sxt[img % 2]
                    # img0's signs gate the post-AllReduce restart: split
                    # top/bottom so conv2 chunks 0-2 start after the top
                    # rows are thresholded instead of the whole image
                    parts = (((0, 32), (32, H))
                             if img == 0 and EARLY_IMG0 and mode == "fp8"
                             else ((0, H),))
                    for part, (r0, r1) in enumerate(parts):
                        for pc in range(NPC):
                            srcv = s1[(img, pc)].rearrange(
                                "p (h w) -> p h w", w=W)
                            dst = plane_dst(sh_tile, pc)
                            nc.scalar.activation(
                                dst[:, 1 + r0 : 1 + r1, 1 : 1 + W],
                                srcv[:, r0:r1, :],
                                mybir.ActivationFunctionType.Sign,
                                bias=c1v[pc][:, 0, :], scale=a1v[pc][:, 0, :],
                            )
                        if mode == "fp8":
                            if len(parts) == 1:
                                make_shift_copies(sh_tile)
                            else:
                                make_shift_copies(
                                    sh_tile,
                                    band=(0, 33) if part == 0 else (32, HP),
                                )
                    for pc in range(NPC):
                        s2t = accpool.tile([128, NPIX], F16,
                                           name=f"s2_{img}_{pc}", tag="acc")
                        s2[(img, pc)] = s2t
                        if mode == "bf16":
                            pss = [_emit_conv_bf16(nc, psum_pool, wsb2,
                                                   sh_tile, pc, chunk)
                                   for chunk in range(NCHUNK)]
                        else:
                            # the barrier-gating last groups (pc1 gates the
                            # group-A AllReduce, pc2 group-B) get two chunk
                            # sets so their evacuations overlap their matmuls
                            # (stop-matmuls otherwise all land at group end)
                            csets = ([range(0, 4), range(4, NCHUNK)]
                                     if img == bpc - 1 and pc >= 1 else None)
                            pss = _emit_conv_fp8(nc, psum_pool, w2p_v, w2s_v,
                                                 sh_tile, pc,
                                                 chunk_sets=csets)
                        for chunk in range(NCHUNK):
                            y0 = chunk * CHUNK_ROWS
                            xr = xrpool.tile([128, CHUNK_ROWS, W], F32,
                                             name="xr", tag="xr")
                            nc.sync.dma_start(
                                out=xr,
                                in_=x_d[img, pc * 128 : (pc + 1) * 128,
                                        y0 : y0 + CHUNK_ROWS],
                            )
                            sl = slice(chunk * CHW, (chunk + 1) * CHW)
                            if mode == "bf16":
                                ps_in = pss[chunk][:]
                                xr_in = xr.rearrange("p h w -> p (h w)")
                                out_ap = s2t[:, sl]
                                nc.vector.tensor_tensor(
                                    out=out_ap, in0=ps_in, in1=xr_in,
                                    op=mybir.AluOpType.add,
                                )
                            else:
                                # evacuation spread over three engines:
                                # scalar strips the seam out of PSUM, gpsimd
                                # adds the residual (SBUF-only engine), DVE
                                # just does bn_stats -- keeps the DVE tail
                                # off the stats chain's critical path
                                ps_in = (pss[chunk]
                                         .rearrange("p (r c) -> p r c", c=WP)
                                         [:, :, 0:W])
                                out_ap = s2t[:, sl].rearrange(
                                    "p (r c) -> p r c", c=W)
                                nc.scalar.copy(out_ap, ps_in)
                                nc.gpsimd.tensor_tensor(
                                    out=out_ap, in0=out_ap, in1=xr[:],
                                    op=mybir.AluOpType.add,
                                )
                            nc.vector.bn_stats(
                                out=bnst2[pc][:, img * NCHUNK + chunk, :],
                                in_=s2t[:, sl],
                            )
                        if img == bpc - 1:
                            for gi, (p0, p1) in enumerate(AR_GROUPS2):
                                if pc == p1 - 1:
                                    cout2[gi] = emit_group_stats(
                                        bnst2, list(range(p0, p1)),
                                        f"bn2{gi}")

                # ---- bn2 finish + pass C, AR-group at a time: group A's
                # scale/bias lands before the group-B AllReduce completes, so
                # 2/3 of the output DMA overlaps it. Emitting finish(B) after
                # group A's stores also keeps its DRAM readback from
                # head-of-line-blocking them in the HWDGE FIFO. ----
                a2v, c2v = [None] * NPC, [None] * NPC

                def emit_pass_c(img, pc):
                    s2t = s2[(img, pc)]
                    for chunk in range(NCHUNK):
                        sl = slice(chunk * CHW, (chunk + 1) * CHW)
                        oc = ocpool.tile([128, CHW], F32, name="oc",
                                         tag="oc")
                        ocs = oc[:]
                        if chunk % 2 == 0:
                            nc.scalar.activation(
                                ocs, s2t[:, sl],
                                mybir.ActivationFunctionType.Identity,
                                bias=c2v[pc][:, 0, :], scale=a2v[pc][:, 0, :],
                            )
                            nc.vector.tensor_scalar(
                                out=ocs, in0=ocs, scalar1=1.0,
                                scalar2=-1.0, op0=mybir.AluOpType.min,
                                op1=mybir.AluOpType.max,
                            )
                        else:
                            nc.vector.tensor_scalar(
                                out=ocs, in0=s2t[:, sl],
                                scalar1=a2v[pc][:, 0, :],
                                scalar2=c2v[pc][:, 0, :],
                                op0=mybir.AluOpType.mult,
                                op1=mybir.AluOpType.add,
                            )
                            nc.gpsimd.tensor_scalar(
                                out=ocs, in0=ocs, scalar1=1.0,
                                scalar2=-1.0, op0=mybir.AluOpType.min,
                                op1=mybir.AluOpType.max,
                            )
                        y0 = chunk * CHUNK_ROWS
                        nc.sync.dma_start(
                            out=out_d[img, pc * 128 : (pc + 1) * 128,
                                      y0 : y0 + CHUNK_ROWS],
                            in_=oc.rearrange("p (h w) -> p h w", w=W),
                        )

                for gi, (p0, p1) in enumerate(AR_GROUPS2):
                    a, c = finish_group_stats(*cout2[gi], 2, 3, f"bn2{gi}",
                                              p0)
                    for k, pc in enumerate(range(p0, p1)):
                        a2v[pc] = a[:, k : k + 1, :]
                        c2v[pc] = c[:, k : k + 1, :]
                    for img in range(bpc):
                        for pc in range(p0, p1):
                            emit_pass_c(img, pc)

    nc.compile()
    return nc


_PROGRAM = None


def _get_program():
    global _PROGRAM
    if _PROGRAM is None:
        _PROGRAM = build_program()
    return _PROGRAM


def make_in_maps(x, W1, W2, g1, b1, g2, b2, bpc=BPC, ncores=NCORES,
                 mode=MODE):
    vecs = _prep_vecs(np.asarray(g1), np.asarray(b1), np.asarray(g2),
                      np.asarray(b2))
    x = np.ascontiguousarray(np.asarray(x, dtype=np.float32))
    if mode == "bf16":
        wmap = {"w1": _prep_weight(np.asarray(W1)),
                "w2": _prep_weight(np.asarray(W2))}
    else:
        w1p, w1s = _prep_weight_fp8(np.asarray(W1))
        w2p, w2s = _prep_weight_fp8(np.asarray(W2))
        wmap = {"w1p": w1p, "w1s": w1s, "w2p": w2p, "w2s": w2s}
    return [
        {"x": x[core * bpc : (core + 1) * bpc], "vecs": vecs, **wmap}
        for core in range(ncores)
    ]


def kernel(x, W1, W2, g1, b1, g2, b2, trace=False):
    nc = _get_program()
    in_maps = make_in_maps(x, W1, W2, g1, b1, g2, b2)
    res = run_bass_kernel_spmd(
        nc, in_maps, core_ids=list(range(NCORES)), trace=trace
    )
    out = np.concatenate([res.results[c]["out"] for c in range(NCORES)], axis=0)
    kernel.last_results = res
    return out
